# revision 3
# baseline (speedup 1.0000x reference)
"""Trainium2 Bass kernel for the box-smoothed Charbonnier loss.

reference:  diff = conv7x7_box(sum_ch(x - y)) / 49 ;  loss = mean(sqrt(diff^2 + 1e-6))

Strategy (pure data parallel, 2 images per core on 8 cores):
  - DMA-accumulate the 3 channels of x (and y) while loading -> channel sums
    land in SBUF with zero compute-engine work; one DVE subtract gives
    s = sum_ch(x - y) per image as a [128, 4, 512] tile (rows chunked by 128).
  - 7-wide box conv in each direction is a banded-matrix matmul on the PE.
    Using the band as the *moving* operand and image data as the stationary
    operand computes conv and transpose in one pass:
        stage1[m, n] = sum_r s[r, 128*cb + m] * band(r, n)     (vertical conv, transposed out)
        stage2[m, n] = sum_w t[w, 128*hb + m] * band(w, n)     (horizontal conv, back to [h, w])
  - Charbonnier on ACT: Square (PSUM->SBUF) then Sqrt(x + eps) with accum_out
    producing per-partition sums; final cross-partition sum via a ones-matmul.
  - Host sums the 8 per-core partial sums and divides by the element count.
"""

import numpy as np

import concourse.bass as bass
import concourse.bacc as bacc
import concourse.mybir as mybir
import concourse.tile as tile
from concourse.bass_interp import get_hw_module
from concourse.bass_utils import run_bass_kernel_spmd

N_CORES = 8
B_TOTAL = 16
B_PER_CORE = B_TOTAL // N_CORES
CH = 3
H = W = 512
P = 128
NCHUNK = H // P  # 4
EPS = 1e-6
F32 = mybir.dt.float32
AF = mybir.ActivationFunctionType


def make_band() -> np.ndarray:
    """band[p, c, n] = 1/7 if |128*c + p - n| <= 3 else 0  (shape [128, 4, 512])."""
    band = np.zeros((P, NCHUNK, W), dtype=np.float32)
    r = np.arange(P)[:, None, None] + P * np.arange(NCHUNK)[None, :, None]
    n = np.arange(W)[None, None, :]
    band[np.abs(r - n) <= 3] = np.float32(1.0) / np.float32(7.0)
    return band


def build_program() -> tuple[bacc.Bacc, str, str, str, str]:
    nc = bacc.Bacc("TRN2", target_bir_lowering=False, debug=False, num_devices=N_CORES)

    x = nc.dram_tensor("x", [B_PER_CORE, CH, H, W], F32, kind="ExternalInput")
    y = nc.dram_tensor("y", [B_PER_CORE, CH, H, W], F32, kind="ExternalInput")
    band = nc.dram_tensor("band", [P, NCHUNK, W], F32, kind="ExternalInput")
    out = nc.dram_tensor("out", [1, 1], F32, kind="ExternalOutput")

    add = mybir.AluOpType.add
    bypass = mybir.AluOpType.bypass

    with tile.TileContext(nc) as tc:
        with (
            tc.tile_pool(name="const", bufs=1) as cpool,
            tc.tile_pool(name="data", bufs=2) as dpool,
            tc.tile_pool(name="small", bufs=2) as spool,
            tc.tile_pool(name="psum", bufs=2, space="PSUM") as ppool,
            tc.tile_pool(name="psum1", bufs=1, space="PSUM") as ppool1,
        ):
            band_t = cpool.tile([P, NCHUNK, W], F32)
            nc.sync.dma_start(band_t[:], band.ap()[:])

            ones = cpool.tile([P, 1], F32)
            nc.gpsimd.memset(ones[:], 1.0)

            epsb = cpool.tile([P, 1], F32)
            nc.gpsimd.memset(epsb[:], float(EPS))

            acc = cpool.tile([P, B_PER_CORE * NCHUNK], F32)

            for b in range(B_PER_CORE):
                sx = dpool.tile([P, NCHUNK, W], F32, tag="sx")
                sy = dpool.tile([P, NCHUNK, W], F32, tag="sy")
                for ch in range(CH):
                    op = bypass if ch == 0 else add
                    nc.gpsimd.dma_start(
                        sx[:],
                        x.ap()[b, ch].rearrange("(c p) w -> p c w", p=P),
                        accum_op=op,
                    )
                    nc.gpsimd.dma_start(
                        sy[:],
                        y.ap()[b, ch].rearrange("(c p) w -> p c w", p=P),
                        accum_op=op,
                    )
                s = dpool.tile([P, NCHUNK, W], F32, tag="s")
                nc.vector.tensor_sub(s[:], sx[:], sy[:])

                # stage 1: vertical conv + transpose, per 128-col block
                t = dpool.tile([P, NCHUNK, W], F32, tag="t")
                for cb in range(NCHUNK):
                    ps1 = ppool.tile([P, W], F32, tag="ps1")
                    for c in range(NCHUNK):
                        nc.tensor.matmul(
                            ps1[:],
                            s[:, c, cb * P:(cb + 1) * P],
                            band_t[:, c, :],
                            start=(c == 0),
                            stop=(c == NCHUNK - 1),
                        )
                    nc.scalar.copy(t[:, cb, :], ps1[:])

                # stage 2: horizontal conv + transpose back, then charbonnier
                for hb in range(NCHUNK):
                    ps2 = ppool.tile([P, W], F32, tag="ps2")
                    for cb in range(NCHUNK):
                        nc.tensor.matmul(
                            ps2[:],
                            t[:, cb, hb * P:(hb + 1) * P],
                            band_t[:, cb, :],
                            start=(cb == 0),
                            stop=(cb == NCHUNK - 1),
                        )
                    sq = spool.tile([P, W], F32, tag="sq")
                    nc.scalar.activation(sq[:], ps2[:], AF.Square)
                    u = spool.tile([P, W], F32, tag="u")
                    col = b * NCHUNK + hb
                    nc.scalar.activation(
                        u[:], sq[:], AF.Sqrt, bias=epsb[:],
                        accum_out=acc[:, col:col + 1],
                    )

            # total = sum over partitions of (sum over the 8 accum columns)
            red = cpool.tile([P, 1], F32)
            nc.vector.tensor_reduce(
                red[:], acc[:], axis=mybir.AxisListType.X, op=add
            )
            ps3 = ppool1.tile([1, 1], F32, tag="ps3")
            nc.tensor.matmul(ps3[:], red[:], ones[:], start=True, stop=True)
            res = cpool.tile([1, 1], F32)
            nc.scalar.copy(res[:], ps3[:])
            nc.sync.dma_start(out.ap()[:], res[:])

    nc.compile()
    nc.m = get_hw_module(nc.m)
    return nc, x.name, y.name, band.name, out.name


_CACHE = {}


def _get_program():
    if "prog" not in _CACHE:
        _CACHE["prog"] = build_program()
    return _CACHE["prog"]


def run_sharded(x: np.ndarray, y: np.ndarray, trace: bool = False):
    """Run the SPMD kernel; returns (per-core sums list, BassKernelResults)."""
    nc, xname, yname, bandname, outname = _get_program()
    band = make_band()
    x = np.ascontiguousarray(np.asarray(x, dtype=np.float32))
    y = np.ascontiguousarray(np.asarray(y, dtype=np.float32))
    in_maps = []
    for k in range(N_CORES):
        sl = slice(k * B_PER_CORE, (k + 1) * B_PER_CORE)
        in_maps.append({
            xname: x[sl],
            yname: y[sl],
            bandname: band,
        })
    res = run_bass_kernel_spmd(
        nc, in_maps, core_ids=list(range(N_CORES)), trace=trace
    )
    sums = [float(res.results[k][outname][0, 0]) for k in range(N_CORES)]
    return sums, res


def kernel(x: np.ndarray, y: np.ndarray) -> np.ndarray:
    sums, _ = run_sharded(x, y)
    total = float(np.sum(np.asarray(sums, dtype=np.float64)))
    return np.float32(total / (B_TOTAL * H * W))


# revision 9
# speedup vs baseline: 1.3968x; 1.3968x over previous
"""Trainium2 Bass kernel for the box-smoothed Charbonnier loss.

reference:  diff = conv7x7_box(sum_ch(x - y)) / 49 ;  loss = mean(sqrt(diff^2 + 1e-6))

Strategy (pure data parallel, 2 images per core on 8 cores):
  - Plain HWDGE loads of x[b], y[b] (split into row-halves for earlier
    compute overlap); DVE computes s = sum_ch(x - y) per image.
  - 7-wide box conv in each direction is a banded-matrix matmul on the PE.
    Band rides as the *moving* operand, image data as the stationary one,
    which fuses conv + transpose:
        stage1[m, n] = sum_r s[r, 128*cb + m] * band(r, n)   (vertical conv, transposed)
        stage2[m, n] = sum_w t[w, 128*hb + m] * band(w, n)   (horizontal conv, back to [h, w])
    The first matmul of each PSUM group runs full-width (initializes the
    bank); the rest touch only their ~134-wide band window.
  - Charbonnier on ACT: Square (PSUM->SBUF), Sqrt(x + eps) with accum_out
    giving per-partition sums; cross-partition total via a ones-matmul.
  - Host sums the 8 per-core partials and divides by the element count.
"""

import numpy as np

import concourse.bass as bass
import concourse.bacc as bacc
import concourse.mybir as mybir
import concourse.tile as tile
from concourse.bass_interp import get_hw_module
from concourse.bass_utils import run_bass_kernel_spmd

N_CORES = 8
B_TOTAL = 16
B_PER_CORE = B_TOTAL // N_CORES
CH = 3
H = W = 512
P = 128
NCHUNK = H // P  # 4
EPS = 1e-6
F32 = mybir.dt.float32
AF = mybir.ActivationFunctionType

# band window of chunk c: rows 128c..128c+127 touch cols [128c-3, 128c+131)
def _win(c):
    return max(0, P * c - 3), min(W, P * c + P + 3)


def make_band() -> np.ndarray:
    """band[p, c, n] = 1/7 if |128*c + p - n| <= 3 else 0  (shape [128, 4, 512])."""
    band = np.zeros((P, NCHUNK, W), dtype=np.float32)
    r = np.arange(P)[:, None, None] + P * np.arange(NCHUNK)[None, :, None]
    n = np.arange(W)[None, None, :]
    band[np.abs(r - n) <= 3] = np.float32(1.0) / np.float32(7.0)
    return band


def build_program() -> tuple[bacc.Bacc, str, str, str, str]:
    nc = bacc.Bacc("TRN2", target_bir_lowering=False, debug=False, num_devices=N_CORES)

    x = nc.dram_tensor("x", [B_PER_CORE, CH, H, W], F32, kind="ExternalInput")
    y = nc.dram_tensor("y", [B_PER_CORE, CH, H, W], F32, kind="ExternalInput")
    band = nc.dram_tensor("band", [P, NCHUNK, W], F32, kind="ExternalInput")
    out = nc.dram_tensor("out", [1, 1], F32, kind="ExternalOutput")

    add = mybir.AluOpType.add

    with tile.TileContext(nc) as tc:
        with (
            tc.tile_pool(name="const", bufs=1) as cpool,
            tc.tile_pool(name="xy", bufs=1) as xypool,
            tc.tile_pool(name="data", bufs=2) as dpool,
            tc.tile_pool(name="small", bufs=2) as spool,
            tc.tile_pool(name="psum", bufs=2, space="PSUM") as ppool,
            tc.tile_pool(name="psum1", bufs=1, space="PSUM") as ppool1,
        ):
            band_t = cpool.tile([P, NCHUNK, W], F32)
            nc.sync.dma_start(band_t[:], band.ap()[:])

            ones = cpool.tile([P, 1], F32)
            nc.gpsimd.memset(ones[:], 1.0)
            epsb = cpool.tile([P, 1], F32)
            nc.gpsimd.memset(epsb[:], float(EPS))

            acc = cpool.tile([P, B_PER_CORE * NCHUNK], F32)

            # prefetch all image loads up-front, split into row-halves
            xt, yt = [], []
            for b in range(B_PER_CORE):
                xb = xypool.tile([P, CH, NCHUNK, W], F32, tag=f"x{b}")
                yb = xypool.tile([P, CH, NCHUNK, W], F32, tag=f"y{b}")
                for ch in range(CH):
                    nc.sync.dma_start(
                        xb[:, ch, :, :],
                        x.ap()[b, ch].rearrange("(c p) w -> p c w", p=P),
                    )
                    nc.sync.dma_start(
                        yb[:, ch, :, :],
                        y.ap()[b, ch].rearrange("(c p) w -> p c w", p=P),
                    )
                xt.append(xb)
                yt.append(yb)

            for b in range(B_PER_CORE):
                xb, yb = xt[b], yt[b]
                # s = sum_ch (x - y): one big sub + two adds per half
                d = xypool.tile([P, CH, NCHUNK, W], F32, tag="d")
                s = dpool.tile([P, NCHUNK, W], F32, tag="s")
                for ch in range(CH):
                    nc.vector.tensor_sub(d[:, ch, :, :], xb[:, ch, :, :], yb[:, ch, :, :])
                nc.vector.tensor_add(s[:], d[:, 0, :, :], d[:, 1, :, :])
                nc.vector.tensor_add(s[:], s[:], d[:, 2, :, :])

                # stage 1: vertical conv + transpose, per 128-col block
                t = dpool.tile([P, NCHUNK, W], F32, tag="t")
                for cb in range(NCHUNK):
                    ps1 = ppool.tile([P, W], F32, tag="ps1")
                    for i, c in enumerate(range(NCHUNK)):
                        lo, hi = (0, W) if i == 0 else _win(c)
                        nc.tensor.matmul(
                            ps1[:, lo:hi],
                            s[:, c, cb * P:(cb + 1) * P],
                            band_t[:, c, lo:hi],
                            start=(i == 0),
                            stop=(i == NCHUNK - 1),
                        )
                    nc.scalar.copy(t[:, cb, :], ps1[:])

                # stage 2: horizontal conv + transpose back, then charbonnier
                for hb in range(NCHUNK):
                    ps2 = ppool.tile([P, W], F32, tag="ps2")
                    for i, cb in enumerate(range(NCHUNK)):
                        lo, hi = (0, W) if i == 0 else _win(cb)
                        nc.tensor.matmul(
                            ps2[:, lo:hi],
                            t[:, cb, hb * P:(hb + 1) * P],
                            band_t[:, cb, lo:hi],
                            start=(i == 0),
                            stop=(i == NCHUNK - 1),
                        )
                    sq = spool.tile([P, W], F32, tag="sq")
                    nc.scalar.activation(sq[:], ps2[:], AF.Square)
                    u = spool.tile([P, W], F32, tag="u")
                    col = b * NCHUNK + hb
                    nc.scalar.activation(
                        u[:], sq[:], AF.Sqrt, bias=epsb[:],
                        accum_out=acc[:, col:col + 1],
                    )

            # total = sum over partitions of (sum over the 8 accum columns)
            red = cpool.tile([P, 1], F32)
            nc.vector.tensor_reduce(
                red[:], acc[:], axis=mybir.AxisListType.X, op=add
            )
            ps3 = ppool1.tile([1, 1], F32, tag="ps3")
            nc.tensor.matmul(ps3[:], red[:], ones[:], start=True, stop=True)
            res = cpool.tile([1, 1], F32)
            nc.scalar.copy(res[:], ps3[:])
            nc.sync.dma_start(out.ap()[:], res[:])

    nc.compile()
    nc.m = get_hw_module(nc.m)
    return nc, x.name, y.name, band.name, out.name


_CACHE = {}


def _get_program():
    if "prog" not in _CACHE:
        _CACHE["prog"] = build_program()
    return _CACHE["prog"]


def run_sharded(x: np.ndarray, y: np.ndarray, trace: bool = False):
    """Run the SPMD kernel; returns (per-core sums list, BassKernelResults)."""
    nc, xname, yname, bandname, outname = _get_program()
    band = make_band()
    x = np.ascontiguousarray(np.asarray(x, dtype=np.float32))
    y = np.ascontiguousarray(np.asarray(y, dtype=np.float32))
    in_maps = []
    for k in range(N_CORES):
        sl = slice(k * B_PER_CORE, (k + 1) * B_PER_CORE)
        in_maps.append({
            xname: x[sl],
            yname: y[sl],
            bandname: band,
        })
    res = run_bass_kernel_spmd(
        nc, in_maps, core_ids=list(range(N_CORES)), trace=trace
    )
    sums = [float(res.results[k][outname][0, 0]) for k in range(N_CORES)]
    return sums, res


def kernel(x: np.ndarray, y: np.ndarray) -> np.ndarray:
    sums, _ = run_sharded(x, y)
    total = float(np.sum(np.asarray(sums, dtype=np.float64)))
    return np.float32(total / (B_TOTAL * H * W))
